# revision 1
# baseline (speedup 1.0000x reference)
"""MoE layer (8 experts, top-2) on 8 Trainium2 NeuronCores, expert-parallel.

Strategy
--------
Host (dispatch): compute router logits/top-k on host, gather each expert's
tokens into a padded capacity buffer (C = max expert load, 4-aligned),
pre-pack activations/weights into the exact SBUF tile layout
(partition-major) in fp16.
Device (one expert per core, SPMD): Y_e = w_down[e] @ (silu(w_gate[e] @ x_e)
* (w_up[e] @ x_e)) over the expert's C gathered tokens; all matmuls fp16
inputs with fp32 PSUM accumulation (fp16 runs at full PE rate like bf16 but
with 8x finer mantissa). Token columns are processed in 512-wide blocks;
the remainder is merged into the last block's weight pass and the merged
pair rebalanced so no block drops below the ~69-col instruction-issue
floor of the PE (measured: the tensor engine issues matmuls no faster
than ~29ns apart, and a 512-col fp16 matmul takes 216ns = 2.37GHz).
Host (combine): scatter-add per-token routing-weighted outputs.

Trace-derived tuning notes (this exact workload, TRN2):
- Only sync (qSPDynamicHW) and scalar (qActDynamicHW) issue HW-DGE DMAs;
  gpsimd DMA is software-DGE and far too slow for streaming.
- All heavy weight streams must ride sync: scalar runs the ACTIVATE
  (silu) instructions, and DMA issues blocked on semaphore-slot reuse
  would delay them, stalling PSUM recycling and the PE.
- Every weight pass needs per-m-iter compute >= the sync ring's ~5-6us
  per-m weight delivery, hence all non-tail blocks stay 512 wide.
"""

import os
import numpy as np
from contextlib import ExitStack

H = 2048
I = 5632
E = 8
P = 128
NB = 512  # token block (matmul free dim / PSUM bank)

KH = H // P   # 16  k-tiles over H
MI = I // P   # 44  m-tiles over I

DT = np.float16  # fp16: PE full rate like bf16, 8x finer mantissa


def _superblocks(C):
    """Column groups; a trailing remainder (<NB) is merged into the last
    full block so both share one pass over the weights.

    Matmuls below ~69 cols are bound by the 29ns instruction-issue floor
    (29ns buys 69 cols at 2.37GHz), so a skinny tail wastes PE time.
    Non-tail passes must stay 512 wide: a narrower pass consumes weights
    faster than the single sync HW-DGE ring delivers (~5-6us per m-iter),
    and the weight stream cannot ride the scalar ring without burying the
    ACTIVATE instructions behind blocking DMA waits.  So rebalance only
    inside the merged tail pass: [512, t<128] -> [384+t, 128]."""
    blocks = []
    t = 0
    while t < C:
        blocks.append((t, min(NB, C - t)))
        t += NB
    sbs = [[b] for b in blocks]
    if len(sbs) >= 2 and sbs[-1][0][1] < NB:
        tail = sbs.pop()[0]
        sbs[-1].append(tail)
        (t0, w0), (t1, w1) = sbs[-1]
        if w1 < 128:
            w0n = w0 + w1 - 128
            sbs[-1] = [(t0, w0n), (t0 + w0n, 128)]
    return sbs


def build_program(C, h=H, i_dim=I, sim_safe_act=False):
    """Build the SPMD bass program for one expert over C tokens.

    DRAM I/O layouts (all partition-major, pre-packed on host):
      x  [P, KH, C]        fp16   x[p, k, t]  = token t, hidden 128k+p
      wg [MI, P, KH*P]     fp16   wg[m, p, kf] (kf = k*128+f): w_gate.T tiles
      wu [MI, P, KH*P]     fp16   same for w_up
      wd [KH, P, MI*P]     fp16   w_down.T tiles
      y  [P, KH, C]        bf16   y[p, m2, t] = output hidden 128*m2+p
           (bf16 keeps y's ~0.2% quantization noise far under the 2e-2
            gate and halves the final drain + scalar-ring write traffic)
    """
    from concourse import bacc, tile, mybir

    kh = h // P
    mi = i_dim // P
    bf = mybir.dt.float16
    bf16 = mybir.dt.bfloat16
    f32 = mybir.dt.float32
    Silu = mybir.ActivationFunctionType.Silu

    nc = bacc.Bacc(None)
    X = nc.declare_dram_parameter("x", [P, kh, C], bf, isOutput=False)
    WG = nc.declare_dram_parameter("wg", [mi, P, kh * P], bf, isOutput=False)
    WU = nc.declare_dram_parameter("wu", [mi, P, kh * P], bf, isOutput=False)
    WD = nc.declare_dram_parameter("wd", [kh, P, mi * P], bf, isOutput=False)
    Y = nc.declare_dram_parameter("y", [P, kh, C], bf16, isOutput=True)

    with ExitStack() as ctx:
        tc = ctx.enter_context(tile.TileContext(nc))
        xpool = ctx.enter_context(tc.tile_pool(name="xpool", bufs=2))
        wpool = ctx.enter_context(tc.tile_pool(name="wpool", bufs=6))
        dpool = ctx.enter_context(tc.tile_pool(name="dpool", bufs=4))
        hpool = ctx.enter_context(tc.tile_pool(name="hpool", bufs=1))
        apool = ctx.enter_context(tc.tile_pool(name="apool", bufs=3))
        ypool = ctx.enter_context(tc.tile_pool(name="ypool", bufs=3))
        pg_pool = ctx.enter_context(tc.tile_pool(name="pg", bufs=3, space="PSUM"))
        pu_pool = ctx.enter_context(tc.tile_pool(name="pu", bufs=3, space="PSUM"))
        py_pool = ctx.enter_context(tc.tile_pool(name="py", bufs=2, space="PSUM"))

        first_sb = True
        for sb in _superblocks(C):
            # Only sync (qSPDynamicHW) and scalar (qActDynamicHW) are
            # hardware DGE rings; gpsimd DMA is software-DGE and slow.
            q = kh * P // 4
            pre_wg = pre_wu = None
            x_ts = []
            if first_sb:
                # ---- first superblock: interleave the m=0 weight chunks
                # with the x chunks across both HW rings so the first pg
                # chain starts at the ~12us DMA-latency floor instead of
                # queueing all 16 x chunks ahead of the weights (~19us).
                (t0, tn) = sb[0]
                x_t = xpool.tile([P, kh, tn], bf, tag="x_t0", name="x_t0")
                x_ts.append(x_t)
                pre_wg = wpool.tile([P, kh * P], bf, tag="wg_t")
                pre_wu = wpool.tile([P, kh * P], bf, tag="wu_t")
                for j in range(4):
                    nc.sync.dma_start(pre_wg[:, j * q : (j + 1) * q], WG[0, :, j * q : (j + 1) * q])
                    nc.scalar.dma_start(pre_wu[:, j * q : (j + 1) * q], WU[0, :, j * q : (j + 1) * q])
                    for k in range(4 * j, 4 * j + 4):
                        eng = nc.sync if k % 2 == 0 else nc.scalar
                        eng.dma_start(x_t[:, k, :tn], X[:, k, t0 : t0 + tn])
            else:
                # ---- load X for each column group: kh tiles [P, tn]
                for g, (t0, tn) in enumerate(sb):
                    x_t = xpool.tile([P, kh, tn], bf, tag=f"x_t{g}", name=f"x_t{g}")
                    for k in range(kh):
                        eng = nc.scalar if k % 2 == 0 else nc.sync
                        eng.dma_start(x_t[:, k, :tn], X[:, k, t0 : t0 + tn])
                    x_ts.append(x_t)
            first_sb = False

            # ---- mm1/mm2 + silu*mul -> h (one weight pass for all groups)
            h_ts = [
                hpool.tile([P, mi, sb[g][1]], bf, tag=f"h{g}", name=f"h_t{g}")
                for g in range(len(sb))
            ]
            for m in range(mi):
                if m == 0 and pre_wg is not None:
                    wg_t, wu_t = pre_wg, pre_wu
                else:
                    # all weights on sync: it is the one HW-DGE ring with no
                    # compute duties, so its blocking DMA waits hurt nothing
                    wg_t = wpool.tile([P, kh * P], bf, tag="wg_t")
                    for j in range(4):
                        nc.sync.dma_start(wg_t[:, j * q : (j + 1) * q], WG[m, :, j * q : (j + 1) * q])
                    wu_t = wpool.tile([P, kh * P], bf, tag="wu_t")
                    for j in range(4):
                        nc.sync.dma_start(wu_t[:, j * q : (j + 1) * q], WU[m, :, j * q : (j + 1) * q])

                pgs, pus = [], []
                for g, (t0, tn) in enumerate(sb):
                    pg = pg_pool.tile([P, NB], f32, tag="pg")
                    pgs.append(pg)
                    for k in range(kh):
                        nc.tensor.matmul(
                            pg[:, :tn],
                            wg_t[:, k * P : (k + 1) * P],
                            x_ts[g][:, k, :tn],
                            start=(k == 0),
                            stop=(k == kh - 1),
                        )
                for g, (t0, tn) in enumerate(sb):
                    pu = pu_pool.tile([P, NB], f32, tag="pu")
                    pus.append(pu)
                    for k in range(kh):
                        nc.tensor.matmul(
                            pu[:, :tn],
                            wu_t[:, k * P : (k + 1) * P],
                            x_ts[g][:, k, :tn],
                            start=(k == 0),
                            stop=(k == kh - 1),
                        )
                for g, (t0, tn) in enumerate(sb):
                    pg, pu = pgs[g], pus[g]
                    g_act = apool.tile([P, NB], f32, tag="g_act")
                    if sim_safe_act:
                        # silu(g) = g * sigmoid(g); CoreSim lacks the Silu LUT
                        nc.scalar.activation(
                            g_act[:, :tn],
                            pg[:, :tn],
                            mybir.ActivationFunctionType.Sigmoid,
                        )
                        nc.vector.tensor_mul(g_act[:, :tn], g_act[:, :tn], pg[:, :tn])
                    else:
                        nc.scalar.activation(g_act[:, :tn], pg[:, :tn], Silu)
                    nc.vector.tensor_mul(h_ts[g][:, m, :tn], g_act[:, :tn], pu[:, :tn])

            # ---- mm3 -> y (one weight pass for all groups)
            for m2 in range(kh):
                dhalf = mi * P // 2
                wd_t = dpool.tile([P, mi * P], bf, tag="wd_t")
                nc.sync.dma_start(wd_t[:, :dhalf], WD[m2, :, :dhalf])
                nc.sync.dma_start(wd_t[:, dhalf:], WD[m2, :, dhalf:])
                # tail group first so its drain hides behind the main
                # stream — except on the very last m2, where main-first
                # leaves only the small tail tile's drain exposed at the end
                g_order = list(enumerate(sb))
                if m2 < kh - 1:
                    g_order = list(reversed(g_order))
                for g, (t0, tn) in g_order:
                    py = py_pool.tile([P, NB], f32, tag="py")
                    for k2 in range(mi):
                        nc.tensor.matmul(
                            py[:, :tn],
                            wd_t[:, k2 * P : (k2 + 1) * P],
                            h_ts[g][:, k2, :tn],
                            start=(k2 == 0),
                            stop=(k2 == mi - 1),
                        )
                    y_sb = ypool.tile([P, NB], bf16, tag="y_sb")
                    nc.vector.tensor_copy(y_sb[:, :tn], py[:, :tn])
                    nc.scalar.dma_start(Y[:, m2, t0 : t0 + tn], y_sb[:, :tn])

    nc.compile()
    return nc


def _route(xf, gate_w, top_k):
    """Host router: returns per-expert (token_indices, weights)."""
    logits = xf @ gate_w.T.astype(np.float32)  # [T, E]
    m = logits.max(-1, keepdims=True)
    p = np.exp(logits - m)
    p /= p.sum(-1, keepdims=True)
    k = int(top_k)
    if k >= E:
        top_i = np.tile(np.arange(E), (xf.shape[0], 1))
    else:
        top_i = np.argpartition(-p, k, axis=-1)[:, :k]
    top_w = np.take_along_axis(p, top_i, axis=-1)
    top_w = top_w / top_w.sum(-1, keepdims=True)
    idxs, wts = [], []
    for e in range(E):
        sel = top_i == e  # [T, k]
        tok = np.nonzero(sel.any(-1))[0]
        w = (top_w * sel).sum(-1)[tok].astype(np.float32)
        idxs.append(tok)
        wts.append(w)
    return idxs, wts


def _pack_w1(w):  # [I, H] -> [MI, P, KH*P]; lhsT tile (m,k)[p,f] = w[128m+f, 128k+p]
    return np.ascontiguousarray(
        w.reshape(MI, P, KH, P).transpose(0, 3, 2, 1).reshape(MI, P, KH * P)
    )


def _pack_w3(w):  # [H, I] -> [KH, P, MI*P]; lhsT tile (m2,k2)[p,f] = w[128m2+f, 128k2+p]
    return np.ascontiguousarray(
        w.reshape(KH, P, MI, P).transpose(0, 3, 2, 1).reshape(KH, P, MI * P)
    )


def kernel(x, gate_w, w_gate, w_up, w_down, top_k):
    from concourse.bass_utils import run_bass_kernel_spmd

    x = np.asarray(x, dtype=np.float32)
    gate_w = np.asarray(gate_w, dtype=np.float32)
    w_gate = np.asarray(w_gate, dtype=np.float32)
    w_up = np.asarray(w_up, dtype=np.float32)
    w_down = np.asarray(w_down, dtype=np.float32)
    shape = x.shape
    xf = x.reshape(-1, shape[-1])
    T = xf.shape[0]

    idxs, wts = _route(xf, gate_w, top_k)
    C = max(max(len(ix) for ix in idxs), NB)
    C = ((C + 3) // 4) * 4  # pad only to 4 (8B DMA lines) — C is the roofline

    nc = build_program(C)

    xf_bf = xf.astype(DT)
    in_maps = []
    for e in range(E):
        tok = idxs[e]
        xg = np.zeros((C, H), dtype=DT)
        xg[: len(tok)] = xf_bf[tok]
        # [C, H] -> x[p, k, t] = xg[t, 128k+p]
        xp = np.ascontiguousarray(xg.reshape(C, KH, P).transpose(2, 1, 0))
        in_maps.append(
            {
                "x": xp,
                "wg": _pack_w1(w_gate[e].astype(DT)),
                "wu": _pack_w1(w_up[e].astype(DT)),
                "wd": _pack_w3(w_down[e].astype(DT)),
            }
        )

    trace = bool(os.environ.get("BASS_TRACE"))
    if trace:
        try:
            import antenv.axon_hooks  # noqa: F401  (trace path needs it under axon)
        except ImportError:
            trace = False
            os.environ["BASS_NEVER_TRACE"] = "1"
    res = run_bass_kernel_spmd(nc, in_maps, list(range(E)), trace=trace)
    globals()["LAST_RESULT"] = res

    out = np.zeros((T, H), dtype=np.float32)
    for e in range(E):
        tok = idxs[e]
        y = res.results[e]["y"].astype(np.float32)  # [P, KH, C] bf16 on device
        yt = y.transpose(2, 1, 0).reshape(C, H)[: len(tok)]
        out[tok] += yt * wts[e][:, None]
    return out.reshape(shape)



# revision 12
# speedup vs baseline: 1.0279x; 1.0279x over previous
"""MoE layer (8 experts, top-2) on 8 Trainium2 NeuronCores, expert-parallel.

Strategy
--------
Host (dispatch): compute router logits/top-k on host, gather each expert's
tokens into a padded capacity buffer (C = max expert load, 4-aligned),
pre-pack activations/weights into the exact SBUF tile layout
(partition-major) in fp16.
Device (one expert per core, SPMD): Y_e = w_down[e] @ (silu(w_gate[e] @ x_e)
* (w_up[e] @ x_e)) over the expert's C gathered tokens; all matmuls fp16
inputs with fp32 PSUM accumulation (fp16 runs at full PE rate like bf16 but
with 8x finer mantissa). Token columns are processed in 512-wide blocks;
the remainder is merged into the last block's weight pass and the merged
pair rebalanced so no block drops below the ~69-col instruction-issue
floor of the PE (measured: the tensor engine issues matmuls no faster
than ~29ns apart, and a 512-col fp16 matmul takes 216ns = 2.37GHz).
Host (combine): scatter-add per-token routing-weighted outputs.

Trace-derived tuning notes (this exact workload, TRN2):
- Only sync (qSPDynamicHW) and scalar (qActDynamicHW) issue HW-DGE DMAs;
  gpsimd DMA is software-DGE and far too slow for streaming.
- All heavy weight streams must ride sync: scalar runs the ACTIVATE
  (silu) instructions, and DMA issues blocked on semaphore-slot reuse
  would delay them, stalling PSUM recycling and the PE.
- Every weight pass needs per-m-iter compute >= the sync ring's ~5-6us
  per-m weight delivery, hence all non-tail blocks stay 512 wide.
"""

import os
import numpy as np
from contextlib import ExitStack

H = 2048
I = 5632
E = 8
P = 128
NB = 512  # token block (matmul free dim / PSUM bank)

KH = H // P   # 16  k-tiles over H
MI = I // P   # 44  m-tiles over I

DT = np.float16  # fp16: PE full rate like bf16, 8x finer mantissa

# Partial-fp8 down-projection: the last 2*N3P h-tiles (I-dim) are written as
# e4m3 and consumed by N3P DoubleRow matmuls (2 k-tiles per instruction at
# ~2x the fp16 PE rate).  Each pair saves ~0.9 cyc/col of mm3; measured
# end-to-end rel err on the true inputs: n3p=4 -> 1.62e-2, n3p=5 -> 1.81e-2
# (gate 2e-2).  S3 rescales the fp8 operands into e4m3's sweet range:
# h8 = h/S3 (wu rows pre-scaled by 1/S3 on host), wd8 = S3*wd.
N3P = 4
S3 = 4.0


def _superblocks(C):
    """Column groups; a trailing remainder (<NB) is merged into the last
    full block so both share one pass over the weights.

    Matmuls below ~69 cols are bound by the 29ns instruction-issue floor
    (29ns buys 69 cols at 2.37GHz), so a skinny tail wastes PE time.
    Non-tail passes must stay 512 wide: a narrower pass consumes weights
    faster than the single sync HW-DGE ring delivers (~5-6us per m-iter),
    and the weight stream cannot ride the scalar ring without burying the
    ACTIVATE instructions behind blocking DMA waits.  So rebalance only
    inside the merged tail pass: [512, t<128] -> [384+t, 128]."""
    blocks = []
    t = 0
    while t < C:
        blocks.append((t, min(NB, C - t)))
        t += NB
    sbs = [[b] for b in blocks]
    if len(sbs) >= 2 and sbs[-1][0][1] < NB:
        tail = sbs.pop()[0]
        sbs[-1].append(tail)
        (t0, w0), (t1, w1) = sbs[-1]
        if w1 < 128:
            w0n = w0 + w1 - 128
            sbs[-1] = [(t0, w0n), (t0 + w0n, 128)]
    return sbs


def build_program(C, h=H, i_dim=I, sim_safe_act=False, n3p=N3P):
    """Build the SPMD bass program for one expert over C tokens.

    DRAM I/O layouts (all partition-major, pre-packed on host):
      x  [P, KH, C]        fp16   x[p, k, t]  = token t, hidden 128k+p
      wg [MI, P, KH*P]     fp16   wg[m, p, kf] (kf = k*128+f): w_gate.T tiles
      wu [MI, P, KH*P]     fp16   same for w_up
      wd [KH, P, MI*P]     fp16   w_down.T tiles
      y  [P, KH, C]        bf16   y[p, m2, t] = output hidden 128*m2+p
           (bf16 keeps y's ~0.2% quantization noise far under the 2e-2
            gate and halves the final drain + scalar-ring write traffic)
    """
    from concourse import bacc, tile, mybir

    kh = h // P
    mi = i_dim // P
    mf16 = mi - 2 * n3p  # h-tiles kept in fp16; the rest are e4m3 DR pairs
    bf = mybir.dt.float16
    bf16 = mybir.dt.bfloat16
    f8 = mybir.dt.float8e4
    f32 = mybir.dt.float32
    Silu = mybir.ActivationFunctionType.Silu
    DR = mybir.MatmulPerfMode.DoubleRow

    nc = bacc.Bacc(None)
    X = nc.declare_dram_parameter("x", [P, kh, C], bf, isOutput=False)
    WG = nc.declare_dram_parameter("wg", [mi, P, kh * P], bf, isOutput=False)
    WU = nc.declare_dram_parameter("wu", [mi, P, kh * P], bf, isOutput=False)
    WD = nc.declare_dram_parameter("wd", [kh, P, mf16 * P], bf, isOutput=False)
    if n3p:
        WD8 = nc.declare_dram_parameter("wd8", [kh, P, 2 * n3p, P], f8, isOutput=False)
    Y = nc.declare_dram_parameter("y", [P, kh, C], bf16, isOutput=True)

    with ExitStack() as ctx:
        tc = ctx.enter_context(tile.TileContext(nc))
        xpool = ctx.enter_context(tc.tile_pool(name="xpool", bufs=2))
        wpool = ctx.enter_context(tc.tile_pool(name="wpool", bufs=6))
        dpool = ctx.enter_context(tc.tile_pool(name="dpool", bufs=4))
        d8pool = ctx.enter_context(tc.tile_pool(name="d8pool", bufs=4)) if n3p else None
        hpool = ctx.enter_context(tc.tile_pool(name="hpool", bufs=1))
        apool = ctx.enter_context(tc.tile_pool(name="apool", bufs=3))
        ypool = ctx.enter_context(tc.tile_pool(name="ypool", bufs=3))
        pg_pool = ctx.enter_context(tc.tile_pool(name="pg", bufs=3, space="PSUM"))
        pu_pool = ctx.enter_context(tc.tile_pool(name="pu", bufs=3, space="PSUM"))
        py_pool = ctx.enter_context(tc.tile_pool(name="py", bufs=2, space="PSUM"))

        first_sb = True
        for sb in _superblocks(C):
            # Only sync (qSPDynamicHW) and scalar (qActDynamicHW) are
            # hardware DGE rings; gpsimd DMA is software-DGE and slow.
            q = kh * P // 4
            pre_wg = pre_wu = None
            x_ts = []
            if first_sb:
                # ---- first superblock: interleave the m=0 weight chunks
                # with the x chunks across both HW rings so the first pg
                # chain starts at the ~12us DMA-latency floor instead of
                # queueing all 16 x chunks ahead of the weights (~19us).
                (t0, tn) = sb[0]
                x_t = xpool.tile([P, kh, tn], bf, tag="x_t0", name="x_t0")
                x_ts.append(x_t)
                pre_wg = wpool.tile([P, kh * P], bf, tag="wg_t")
                pre_wu = wpool.tile([P, kh * P], bf, tag="wu_t")
                for j in range(4):
                    nc.sync.dma_start(pre_wg[:, j * q : (j + 1) * q], WG[0, :, j * q : (j + 1) * q])
                    nc.scalar.dma_start(pre_wu[:, j * q : (j + 1) * q], WU[0, :, j * q : (j + 1) * q])
                    for k in range(4 * j, 4 * j + 4):
                        eng = nc.sync if k % 2 == 0 else nc.scalar
                        eng.dma_start(x_t[:, k, :tn], X[:, k, t0 : t0 + tn])
            else:
                # ---- load X for each column group: kh tiles [P, tn]
                for g, (t0, tn) in enumerate(sb):
                    x_t = xpool.tile([P, kh, tn], bf, tag=f"x_t{g}", name=f"x_t{g}")
                    for k in range(kh):
                        eng = nc.scalar if k % 2 == 0 else nc.sync
                        eng.dma_start(x_t[:, k, :tn], X[:, k, t0 : t0 + tn])
                    x_ts.append(x_t)
            first_sb = False

            # ---- mm1/mm2 + silu*mul -> h (one weight pass for all groups)
            h_ts = [
                hpool.tile([P, mf16, sb[g][1]], bf, tag=f"h{g}", name=f"h_t{g}")
                for g in range(len(sb))
            ]
            h8_ts = [
                hpool.tile([P, 2 * n3p, sb[g][1]], f8, tag=f"h8{g}", name=f"h8_t{g}")
                for g in range(len(sb))
            ] if n3p else None
            for m in range(mi):
                if m == 0 and pre_wg is not None:
                    wg_t, wu_t = pre_wg, pre_wu
                else:
                    # all weights on sync: it is the one HW-DGE ring with no
                    # compute duties, so its blocking DMA waits hurt nothing
                    wg_t = wpool.tile([P, kh * P], bf, tag="wg_t")
                    for j in range(4):
                        nc.sync.dma_start(wg_t[:, j * q : (j + 1) * q], WG[m, :, j * q : (j + 1) * q])
                    wu_t = wpool.tile([P, kh * P], bf, tag="wu_t")
                    for j in range(4):
                        nc.sync.dma_start(wu_t[:, j * q : (j + 1) * q], WU[m, :, j * q : (j + 1) * q])

                pgs, pus = [], []
                for g, (t0, tn) in enumerate(sb):
                    pg = pg_pool.tile([P, NB], f32, tag="pg")
                    pgs.append(pg)
                    for k in range(kh):
                        nc.tensor.matmul(
                            pg[:, :tn],
                            wg_t[:, k * P : (k + 1) * P],
                            x_ts[g][:, k, :tn],
                            start=(k == 0),
                            stop=(k == kh - 1),
                        )
                for g, (t0, tn) in enumerate(sb):
                    pu = pu_pool.tile([P, NB], f32, tag="pu")
                    pus.append(pu)
                    for k in range(kh):
                        nc.tensor.matmul(
                            pu[:, :tn],
                            wu_t[:, k * P : (k + 1) * P],
                            x_ts[g][:, k, :tn],
                            start=(k == 0),
                            stop=(k == kh - 1),
                        )
                for g, (t0, tn) in enumerate(sb):
                    pg, pu = pgs[g], pus[g]
                    g_act = apool.tile([P, NB], f32, tag="g_act")
                    if sim_safe_act:
                        # silu(g) = g * sigmoid(g); CoreSim lacks the Silu LUT
                        nc.scalar.activation(
                            g_act[:, :tn],
                            pg[:, :tn],
                            mybir.ActivationFunctionType.Sigmoid,
                        )
                        nc.vector.tensor_mul(g_act[:, :tn], g_act[:, :tn], pg[:, :tn])
                    else:
                        nc.scalar.activation(g_act[:, :tn], pg[:, :tn], Silu)
                    if m < mf16:
                        h_dst = h_ts[g][:, m, :tn]
                    else:
                        # wu rows for these m-tiles are pre-scaled by 1/S3 on
                        # the host, so this writes h/S3 straight as e4m3
                        h_dst = h8_ts[g][:, m - mf16, :tn]
                    nc.vector.tensor_mul(h_dst, g_act[:, :tn], pu[:, :tn])

            # ---- mm3 -> y (one weight pass for all groups)
            for m2 in range(kh):
                dhalf = mf16 * P // 2
                wd_t = dpool.tile([P, mf16 * P], bf, tag="wd_t")
                nc.sync.dma_start(wd_t[:, :dhalf], WD[m2, :, :dhalf])
                nc.sync.dma_start(wd_t[:, dhalf:], WD[m2, :, dhalf:])
                if n3p:
                    wd8_t = d8pool.tile([P, 2 * n3p, P], f8, tag="wd8_t")
                    nc.sync.dma_start(wd8_t[:, :, :], WD8[m2])
                # tail group first so its drain hides behind the main
                # stream — except on the very last m2, where main-first
                # leaves only the small tail tile's drain exposed at the end
                g_order = list(enumerate(sb))
                if m2 < kh - 1:
                    g_order = list(reversed(g_order))
                for g, (t0, tn) in g_order:
                    py = py_pool.tile([P, NB], f32, tag="py")
                    for k2 in range(mf16):
                        nc.tensor.matmul(
                            py[:, :tn],
                            wd_t[:, k2 * P : (k2 + 1) * P],
                            h_ts[g][:, k2, :tn],
                            start=(k2 == 0),
                            stop=(k2 == mf16 - 1 and not n3p),
                        )
                    for j in range(n3p):
                        nc.tensor.matmul(
                            py[:, :tn],
                            wd8_t[:, 2 * j : 2 * j + 2, :],
                            h8_ts[g][:, 2 * j : 2 * j + 2, :tn],
                            start=False,
                            stop=(j == n3p - 1),
                            perf_mode=DR,
                        )
                    y_sb = ypool.tile([P, NB], bf16, tag="y_sb")
                    nc.vector.tensor_copy(y_sb[:, :tn], py[:, :tn])
                    nc.scalar.dma_start(Y[:, m2, t0 : t0 + tn], y_sb[:, :tn])

    nc.compile()
    return nc


def _route(xf, gate_w, top_k):
    """Host router: returns per-expert (token_indices, weights)."""
    logits = xf @ gate_w.T.astype(np.float32)  # [T, E]
    m = logits.max(-1, keepdims=True)
    p = np.exp(logits - m)
    p /= p.sum(-1, keepdims=True)
    k = int(top_k)
    if k >= E:
        top_i = np.tile(np.arange(E), (xf.shape[0], 1))
    else:
        top_i = np.argpartition(-p, k, axis=-1)[:, :k]
    top_w = np.take_along_axis(p, top_i, axis=-1)
    top_w = top_w / top_w.sum(-1, keepdims=True)
    idxs, wts = [], []
    for e in range(E):
        sel = top_i == e  # [T, k]
        tok = np.nonzero(sel.any(-1))[0]
        w = (top_w * sel).sum(-1)[tok].astype(np.float32)
        idxs.append(tok)
        wts.append(w)
    return idxs, wts


def _pack_w1(w):  # [I, H] -> [MI, P, KH*P]; lhsT tile (m,k)[p,f] = w[128m+f, 128k+p]
    return np.ascontiguousarray(
        w.reshape(MI, P, KH, P).transpose(0, 3, 2, 1).reshape(MI, P, KH * P)
    )


def _pack_w3(w):  # [H, I16] -> [KH, P, MF*P]; lhsT tile (m2,k2)[p,f] = w[128m2+f, 128k2+p]
    mf = w.shape[1] // P
    return np.ascontiguousarray(
        w.reshape(KH, P, mf, P).transpose(0, 3, 2, 1).reshape(KH, P, mf * P)
    )


def _pack_w3_f8(w, n3p, s3):
    """[H, 2*n3p*P] fp8 region of w_down -> [KH, P, 2*n3p, P] e4m3 (DR pairs).

    wd8[m2, p, 2j+i, f] = s3 * w[128*m2+f, 128*(2j+i)+p]; the DoubleRow lhsT
    view [:, 2j:2j+2, :] then pairs slot i with the rhs h8 tile 2j+i."""
    import ml_dtypes

    q = (w * s3).astype(ml_dtypes.float8_e4m3)
    return np.ascontiguousarray(
        q.reshape(KH, P, 2 * n3p, P).transpose(0, 3, 2, 1)
    )


def kernel(x, gate_w, w_gate, w_up, w_down, top_k):
    from concourse.bass_utils import run_bass_kernel_spmd

    x = np.asarray(x, dtype=np.float32)
    gate_w = np.asarray(gate_w, dtype=np.float32)
    w_gate = np.asarray(w_gate, dtype=np.float32)
    w_up = np.asarray(w_up, dtype=np.float32)
    w_down = np.asarray(w_down, dtype=np.float32)
    shape = x.shape
    xf = x.reshape(-1, shape[-1])
    T = xf.shape[0]

    idxs, wts = _route(xf, gate_w, top_k)
    C = max(max(len(ix) for ix in idxs), NB)
    C = ((C + 3) // 4) * 4  # pad only to 4 (8B DMA lines) — C is the roofline

    nc = build_program(C)

    xf_bf = xf.astype(DT)
    mcut = (MI - 2 * N3P) * P  # I-dim boundary: below fp16 h, above e4m3 h
    in_maps = []
    for e in range(E):
        tok = idxs[e]
        xg = np.zeros((C, H), dtype=DT)
        xg[: len(tok)] = xf_bf[tok]
        # [C, H] -> x[p, k, t] = xg[t, 128k+p]
        xp = np.ascontiguousarray(xg.reshape(C, KH, P).transpose(2, 1, 0))
        wu_e = w_up[e].copy()
        if N3P:
            wu_e[mcut:] *= 1.0 / S3  # device then writes h/S3 straight as e4m3
        im = {
            "x": xp,
            "wg": _pack_w1(w_gate[e].astype(DT)),
            "wu": _pack_w1(wu_e.astype(DT)),
            "wd": _pack_w3(w_down[e][:, :mcut].astype(DT)),
        }
        if N3P:
            im["wd8"] = _pack_w3_f8(w_down[e][:, mcut:], N3P, S3)
        in_maps.append(im)

    trace = bool(os.environ.get("BASS_TRACE"))
    if trace:
        try:
            import antenv.axon_hooks  # noqa: F401  (trace path needs it under axon)
        except ImportError:
            trace = False
            os.environ["BASS_NEVER_TRACE"] = "1"
    res = run_bass_kernel_spmd(nc, in_maps, list(range(E)), trace=trace)
    globals()["LAST_RESULT"] = res

    out = np.zeros((T, H), dtype=np.float32)
    for e in range(E):
        tok = idxs[e]
        y = res.results[e]["y"].astype(np.float32)  # [P, KH, C] bf16 on device
        yt = y.transpose(2, 1, 0).reshape(C, H)[: len(tok)]
        out[tok] += yt * wts[e][:, None]
    return out.reshape(shape)



# revision 13
# speedup vs baseline: 1.0352x; 1.0071x over previous
"""MoE layer (8 experts, top-2) on 8 Trainium2 NeuronCores, expert-parallel.

Strategy
--------
Host (dispatch): compute router logits/top-k on host, gather each expert's
tokens into a padded capacity buffer (C = max expert load, 4-aligned),
pre-pack activations/weights into the exact SBUF tile layout
(partition-major) in fp16.
Device (one expert per core, SPMD): Y_e = w_down[e] @ (silu(w_gate[e] @ x_e)
* (w_up[e] @ x_e)) over the expert's C gathered tokens; all matmuls fp16
inputs with fp32 PSUM accumulation (fp16 runs at full PE rate like bf16 but
with 8x finer mantissa). Token columns are processed in 512-wide blocks;
the remainder is merged into the last block's weight pass and the merged
pair rebalanced so no block drops below the ~69-col instruction-issue
floor of the PE (measured: the tensor engine issues matmuls no faster
than ~29ns apart, and a 512-col fp16 matmul takes 216ns = 2.37GHz).
Host (combine): scatter-add per-token routing-weighted outputs.

Trace-derived tuning notes (this exact workload, TRN2):
- Only sync (qSPDynamicHW) and scalar (qActDynamicHW) issue HW-DGE DMAs;
  gpsimd DMA is software-DGE and far too slow for streaming.
- All heavy weight streams must ride sync: scalar runs the ACTIVATE
  (silu) instructions, and DMA issues blocked on semaphore-slot reuse
  would delay them, stalling PSUM recycling and the PE.
- Every weight pass needs per-m-iter compute >= the sync ring's ~5-6us
  per-m weight delivery, hence all non-tail blocks stay 512 wide.
"""

import os
import numpy as np
from contextlib import ExitStack

H = 2048
I = 5632
E = 8
P = 128
NB = 512  # token block (matmul free dim / PSUM bank)

KH = H // P   # 16  k-tiles over H
MI = I // P   # 44  m-tiles over I

DT = np.float16  # fp16: PE full rate like bf16, 8x finer mantissa

# Partial-fp8 down-projection: the last 2*N3P h-tiles (I-dim) are written as
# e4m3 and consumed by N3P DoubleRow matmuls (2 k-tiles per instruction at
# ~2x the fp16 PE rate).  Each pair saves ~0.9 cyc/col of mm3; measured
# end-to-end rel err on the true inputs: n3p=4 -> 1.62e-2, n3p=5 -> 1.81e-2
# (gate 2e-2).  S3 rescales the fp8 operands into e4m3's sweet range:
# h8 = h/S3 (wu rows pre-scaled by 1/S3 on host), wd8 = S3*wd.
N3P = 5
S3 = 4.0


def _superblocks(C):
    """Column groups; a trailing remainder (<NB) is merged into the last
    full block so both share one pass over the weights.

    Matmuls below ~69 cols are bound by the 29ns instruction-issue floor
    (29ns buys 69 cols at 2.37GHz), so a skinny tail wastes PE time.
    Non-tail passes must stay 512 wide: a narrower pass consumes weights
    faster than the single sync HW-DGE ring delivers (~5-6us per m-iter),
    and the weight stream cannot ride the scalar ring without burying the
    ACTIVATE instructions behind blocking DMA waits.  So rebalance only
    inside the merged tail pass: [512, t<128] -> [384+t, 128]."""
    blocks = []
    t = 0
    while t < C:
        blocks.append((t, min(NB, C - t)))
        t += NB
    sbs = [[b] for b in blocks]
    if len(sbs) >= 2 and sbs[-1][0][1] < NB:
        tail = sbs.pop()[0]
        sbs[-1].append(tail)
        (t0, w0), (t1, w1) = sbs[-1]
        if w1 < 128:
            w0n = w0 + w1 - 128
            sbs[-1] = [(t0, w0n), (t0 + w0n, 128)]
    return sbs


def build_program(C, h=H, i_dim=I, sim_safe_act=False, n3p=N3P):
    """Build the SPMD bass program for one expert over C tokens.

    DRAM I/O layouts (all partition-major, pre-packed on host):
      x  [P, KH, C]        fp16   x[p, k, t]  = token t, hidden 128k+p
      wg [MI, P, KH*P]     fp16   wg[m, p, kf] (kf = k*128+f): w_gate.T tiles
      wu [MI, P, KH*P]     fp16   same for w_up
      wd [KH, P, MI*P]     fp16   w_down.T tiles
      y  [P, KH, C]        bf16   y[p, m2, t] = output hidden 128*m2+p
           (bf16 keeps y's ~0.2% quantization noise far under the 2e-2
            gate and halves the final drain + scalar-ring write traffic)
    """
    from concourse import bacc, tile, mybir

    kh = h // P
    mi = i_dim // P
    mf16 = mi - 2 * n3p  # h-tiles kept in fp16; the rest are e4m3 DR pairs
    bf = mybir.dt.float16
    bf16 = mybir.dt.bfloat16
    f8 = mybir.dt.float8e4
    f32 = mybir.dt.float32
    Silu = mybir.ActivationFunctionType.Silu
    DR = mybir.MatmulPerfMode.DoubleRow

    nc = bacc.Bacc(None)
    X = nc.declare_dram_parameter("x", [P, kh, C], bf, isOutput=False)
    WG = nc.declare_dram_parameter("wg", [mi, P, kh * P], bf, isOutput=False)
    WU = nc.declare_dram_parameter("wu", [mi, P, kh * P], bf, isOutput=False)
    WD = nc.declare_dram_parameter("wd", [kh, P, mf16 * P], bf, isOutput=False)
    if n3p:
        WD8 = nc.declare_dram_parameter("wd8", [kh, P, 2 * n3p, P], f8, isOutput=False)
    Y = nc.declare_dram_parameter("y", [P, kh, C], bf16, isOutput=True)

    with ExitStack() as ctx:
        tc = ctx.enter_context(tile.TileContext(nc))
        xpool = ctx.enter_context(tc.tile_pool(name="xpool", bufs=2))
        wpool = ctx.enter_context(tc.tile_pool(name="wpool", bufs=6))
        dpool = ctx.enter_context(tc.tile_pool(name="dpool", bufs=4))
        d8pool = ctx.enter_context(tc.tile_pool(name="d8pool", bufs=4)) if n3p else None
        hpool = ctx.enter_context(tc.tile_pool(name="hpool", bufs=1))
        apool = ctx.enter_context(tc.tile_pool(name="apool", bufs=3))
        ypool = ctx.enter_context(tc.tile_pool(name="ypool", bufs=3))
        pg_pool = ctx.enter_context(tc.tile_pool(name="pg", bufs=3, space="PSUM"))
        pu_pool = ctx.enter_context(tc.tile_pool(name="pu", bufs=3, space="PSUM"))
        py_pool = ctx.enter_context(tc.tile_pool(name="py", bufs=2, space="PSUM"))

        first_sb = True
        for sb in _superblocks(C):
            # Only sync (qSPDynamicHW) and scalar (qActDynamicHW) are
            # hardware DGE rings; gpsimd DMA is software-DGE and slow.
            q = kh * P // 4
            pre_wg = pre_wu = None
            x_ts = []
            if first_sb:
                # ---- first superblock: interleave the m=0 weight chunks
                # with the x chunks across both HW rings so the first pg
                # chain starts at the ~12us DMA-latency floor instead of
                # queueing all 16 x chunks ahead of the weights (~19us).
                (t0, tn) = sb[0]
                x_t = xpool.tile([P, kh, tn], bf, tag="x_t0", name="x_t0")
                x_ts.append(x_t)
                pre_wg = wpool.tile([P, kh * P], bf, tag="wg_t")
                pre_wu = wpool.tile([P, kh * P], bf, tag="wu_t")
                for j in range(4):
                    nc.sync.dma_start(pre_wg[:, j * q : (j + 1) * q], WG[0, :, j * q : (j + 1) * q])
                    nc.scalar.dma_start(pre_wu[:, j * q : (j + 1) * q], WU[0, :, j * q : (j + 1) * q])
                    for k in range(4 * j, 4 * j + 4):
                        eng = nc.sync if k % 2 == 0 else nc.scalar
                        eng.dma_start(x_t[:, k, :tn], X[:, k, t0 : t0 + tn])
            else:
                # ---- load X for each column group: kh tiles [P, tn]
                for g, (t0, tn) in enumerate(sb):
                    x_t = xpool.tile([P, kh, tn], bf, tag=f"x_t{g}", name=f"x_t{g}")
                    for k in range(kh):
                        eng = nc.scalar if k % 2 == 0 else nc.sync
                        eng.dma_start(x_t[:, k, :tn], X[:, k, t0 : t0 + tn])
                    x_ts.append(x_t)
            first_sb = False

            # ---- mm1/mm2 + silu*mul -> h (one weight pass for all groups)
            h_ts = [
                hpool.tile([P, mf16, sb[g][1]], bf, tag=f"h{g}", name=f"h_t{g}")
                for g in range(len(sb))
            ]
            h8_ts = [
                hpool.tile([P, 2 * n3p, sb[g][1]], f8, tag=f"h8{g}", name=f"h8_t{g}")
                for g in range(len(sb))
            ] if n3p else None
            for m in range(mi):
                if m == 0 and pre_wg is not None:
                    wg_t, wu_t = pre_wg, pre_wu
                else:
                    # all weights on sync: it is the one HW-DGE ring with no
                    # compute duties, so its blocking DMA waits hurt nothing
                    wg_t = wpool.tile([P, kh * P], bf, tag="wg_t")
                    for j in range(4):
                        nc.sync.dma_start(wg_t[:, j * q : (j + 1) * q], WG[m, :, j * q : (j + 1) * q])
                    wu_t = wpool.tile([P, kh * P], bf, tag="wu_t")
                    for j in range(4):
                        nc.sync.dma_start(wu_t[:, j * q : (j + 1) * q], WU[m, :, j * q : (j + 1) * q])

                pgs, pus = [], []
                for g, (t0, tn) in enumerate(sb):
                    pg = pg_pool.tile([P, NB], f32, tag="pg")
                    pgs.append(pg)
                    for k in range(kh):
                        nc.tensor.matmul(
                            pg[:, :tn],
                            wg_t[:, k * P : (k + 1) * P],
                            x_ts[g][:, k, :tn],
                            start=(k == 0),
                            stop=(k == kh - 1),
                        )
                for g, (t0, tn) in enumerate(sb):
                    pu = pu_pool.tile([P, NB], f32, tag="pu")
                    pus.append(pu)
                    for k in range(kh):
                        nc.tensor.matmul(
                            pu[:, :tn],
                            wu_t[:, k * P : (k + 1) * P],
                            x_ts[g][:, k, :tn],
                            start=(k == 0),
                            stop=(k == kh - 1),
                        )
                for g, (t0, tn) in enumerate(sb):
                    pg, pu = pgs[g], pus[g]
                    g_act = apool.tile([P, NB], f32, tag="g_act")
                    if sim_safe_act:
                        # silu(g) = g * sigmoid(g); CoreSim lacks the Silu LUT
                        nc.scalar.activation(
                            g_act[:, :tn],
                            pg[:, :tn],
                            mybir.ActivationFunctionType.Sigmoid,
                        )
                        nc.vector.tensor_mul(g_act[:, :tn], g_act[:, :tn], pg[:, :tn])
                    else:
                        nc.scalar.activation(g_act[:, :tn], pg[:, :tn], Silu)
                    if m < mf16:
                        h_dst = h_ts[g][:, m, :tn]
                    else:
                        # wu rows for these m-tiles are pre-scaled by 1/S3 on
                        # the host, so this writes h/S3 straight as e4m3
                        h_dst = h8_ts[g][:, m - mf16, :tn]
                    nc.vector.tensor_mul(h_dst, g_act[:, :tn], pu[:, :tn])

            # ---- mm3 -> y (one weight pass for all groups)
            for m2 in range(kh):
                dhalf = mf16 * P // 2
                wd_t = dpool.tile([P, mf16 * P], bf, tag="wd_t")
                nc.sync.dma_start(wd_t[:, :dhalf], WD[m2, :, :dhalf])
                nc.sync.dma_start(wd_t[:, dhalf:], WD[m2, :, dhalf:])
                if n3p:
                    wd8_t = d8pool.tile([P, 2 * n3p, P], f8, tag="wd8_t")
                    nc.sync.dma_start(wd8_t[:, :, :], WD8[m2])
                # tail group first so its drain hides behind the main
                # stream — except on the very last m2, where main-first
                # leaves only the small tail tile's drain exposed at the end
                g_order = list(enumerate(sb))
                if m2 < kh - 1:
                    g_order = list(reversed(g_order))
                for g, (t0, tn) in g_order:
                    py = py_pool.tile([P, NB], f32, tag="py")
                    for k2 in range(mf16):
                        nc.tensor.matmul(
                            py[:, :tn],
                            wd_t[:, k2 * P : (k2 + 1) * P],
                            h_ts[g][:, k2, :tn],
                            start=(k2 == 0),
                            stop=(k2 == mf16 - 1 and not n3p),
                        )
                    for j in range(n3p):
                        nc.tensor.matmul(
                            py[:, :tn],
                            wd8_t[:, 2 * j : 2 * j + 2, :],
                            h8_ts[g][:, 2 * j : 2 * j + 2, :tn],
                            start=False,
                            stop=(j == n3p - 1),
                            perf_mode=DR,
                        )
                    y_sb = ypool.tile([P, NB], bf16, tag="y_sb")
                    nc.vector.tensor_copy(y_sb[:, :tn], py[:, :tn])
                    nc.scalar.dma_start(Y[:, m2, t0 : t0 + tn], y_sb[:, :tn])

    nc.compile()
    return nc


def _route(xf, gate_w, top_k):
    """Host router: returns per-expert (token_indices, weights)."""
    logits = xf @ gate_w.T.astype(np.float32)  # [T, E]
    m = logits.max(-1, keepdims=True)
    p = np.exp(logits - m)
    p /= p.sum(-1, keepdims=True)
    k = int(top_k)
    if k >= E:
        top_i = np.tile(np.arange(E), (xf.shape[0], 1))
    else:
        top_i = np.argpartition(-p, k, axis=-1)[:, :k]
    top_w = np.take_along_axis(p, top_i, axis=-1)
    top_w = top_w / top_w.sum(-1, keepdims=True)
    idxs, wts = [], []
    for e in range(E):
        sel = top_i == e  # [T, k]
        tok = np.nonzero(sel.any(-1))[0]
        w = (top_w * sel).sum(-1)[tok].astype(np.float32)
        idxs.append(tok)
        wts.append(w)
    return idxs, wts


def _pack_w1(w):  # [I, H] -> [MI, P, KH*P]; lhsT tile (m,k)[p,f] = w[128m+f, 128k+p]
    return np.ascontiguousarray(
        w.reshape(MI, P, KH, P).transpose(0, 3, 2, 1).reshape(MI, P, KH * P)
    )


def _pack_w3(w):  # [H, I16] -> [KH, P, MF*P]; lhsT tile (m2,k2)[p,f] = w[128m2+f, 128k2+p]
    mf = w.shape[1] // P
    return np.ascontiguousarray(
        w.reshape(KH, P, mf, P).transpose(0, 3, 2, 1).reshape(KH, P, mf * P)
    )


def _pack_w3_f8(w, n3p, s3):
    """[H, 2*n3p*P] fp8 region of w_down -> [KH, P, 2*n3p, P] e4m3 (DR pairs).

    wd8[m2, p, 2j+i, f] = s3 * w[128*m2+f, 128*(2j+i)+p]; the DoubleRow lhsT
    view [:, 2j:2j+2, :] then pairs slot i with the rhs h8 tile 2j+i."""
    import ml_dtypes

    q = (w * s3).astype(ml_dtypes.float8_e4m3)
    return np.ascontiguousarray(
        q.reshape(KH, P, 2 * n3p, P).transpose(0, 3, 2, 1)
    )


def kernel(x, gate_w, w_gate, w_up, w_down, top_k):
    from concourse.bass_utils import run_bass_kernel_spmd

    x = np.asarray(x, dtype=np.float32)
    gate_w = np.asarray(gate_w, dtype=np.float32)
    w_gate = np.asarray(w_gate, dtype=np.float32)
    w_up = np.asarray(w_up, dtype=np.float32)
    w_down = np.asarray(w_down, dtype=np.float32)
    shape = x.shape
    xf = x.reshape(-1, shape[-1])
    T = xf.shape[0]

    idxs, wts = _route(xf, gate_w, top_k)
    C = max(max(len(ix) for ix in idxs), NB)
    C = ((C + 3) // 4) * 4  # pad only to 4 (8B DMA lines) — C is the roofline

    nc = build_program(C)

    xf_bf = xf.astype(DT)
    mcut = (MI - 2 * N3P) * P  # I-dim boundary: below fp16 h, above e4m3 h
    in_maps = []
    for e in range(E):
        tok = idxs[e]
        xg = np.zeros((C, H), dtype=DT)
        xg[: len(tok)] = xf_bf[tok]
        # [C, H] -> x[p, k, t] = xg[t, 128k+p]
        xp = np.ascontiguousarray(xg.reshape(C, KH, P).transpose(2, 1, 0))
        wu_e = w_up[e].copy()
        if N3P:
            wu_e[mcut:] *= 1.0 / S3  # device then writes h/S3 straight as e4m3
        im = {
            "x": xp,
            "wg": _pack_w1(w_gate[e].astype(DT)),
            "wu": _pack_w1(wu_e.astype(DT)),
            "wd": _pack_w3(w_down[e][:, :mcut].astype(DT)),
        }
        if N3P:
            im["wd8"] = _pack_w3_f8(w_down[e][:, mcut:], N3P, S3)
        in_maps.append(im)

    trace = bool(os.environ.get("BASS_TRACE"))
    if trace:
        try:
            import antenv.axon_hooks  # noqa: F401  (trace path needs it under axon)
        except ImportError:
            trace = False
            os.environ["BASS_NEVER_TRACE"] = "1"
    res = run_bass_kernel_spmd(nc, in_maps, list(range(E)), trace=trace)
    globals()["LAST_RESULT"] = res

    out = np.zeros((T, H), dtype=np.float32)
    for e in range(E):
        tok = idxs[e]
        y = res.results[e]["y"].astype(np.float32)  # [P, KH, C] bf16 on device
        yt = y.transpose(2, 1, 0).reshape(C, H)[: len(tok)]
        out[tok] += yt * wts[e][:, None]
    return out.reshape(shape)

